# revision 2
# baseline (speedup 1.0000x reference)
"""Trainium2 Bass kernel for batched per-item GRU cell (final).

v6 + two structural fixes + fp8 r-gate:

  * ALL aux data (xh columns f32, r/z/c bias blocks) ships as ONE
    contiguous [128, 4*PER] f32 DMA on the SWDGE queue at t=0
    (~5 KB/partition, line-rate) — v6's per-chunk column DMAs
    fragmented into 3300+ sub-512B descriptors and clogged the
    HWDGE rings.  xh is cast to bf16/fp8 on DVE per chunk.
  * The HWDGE rings carry nothing but the weight stream.
  * fp8e4m3 r-gate: r-column weights + the r-matmul moving operand in
    fp8 (error attenuated through sigmoid -> r*h -> c-matmul -> tanh;
    emulated end-to-end 5.8e-3 vs the 2e-2 gate).  z/c stay bf16.
    Weight stream: 20.7 -> 17.0 MB/core.  Three pair-packed PE passes
    per chunk (r fp8, z bf16, c bf16), stride-1 diagonal PSUM outputs.
"""

import numpy as np

import concourse.bass as bass
import concourse.mybir as mybir
import concourse.tile as tile
from concourse import bacc
from concourse.bass_utils import run_bass_kernel_spmd
from concourse.masks import make_identity

F32 = mybir.dt.float32
BF16 = mybir.dt.bfloat16
FP8 = mybir.dt.float8e4

B, N, C, H = 16, 207, 64, 64
J = 3 * H                  # 192
ITEMS = B * N              # 3312
NCORES = 8
PER = ITEMS // NCORES      # 414
CHUNKS = [16, 48, 90, 88, 92, 64, 16]   # sum=414, all even
NCHUNK = len(CHUNKS)
GMAX = max(CHUNKS)
LAST = NCHUNK - 1

AF = mybir.ActivationFunctionType


def build_nc():
    nc = bacc.Bacc(None)
    # one aux stream, 4 f32 [128, G] blocks per chunk:
    # [xh32 | br_diag | bz_diag | bct_dup]
    aux_d = nc.declare_dram_parameter("aux", [128 * 4 * PER], F32,
                                      isOutput=False)
    # fp8 r-pairs: per chunk [c=128, pair, 128] = [r_even | r_odd]
    w8_d = nc.declare_dram_parameter("w8", [PER * 2 * C * 64], FP8,
                                     isOutput=False)
    # bf16 z+c pairs: per chunk [c=128, pair, 256] = [z_e|z_o|c_e|c_o]
    w16_d = nc.declare_dram_parameter("w16", [PER * 2 * C * 128], BF16,
                                      isOutput=False)
    out_d = nc.declare_dram_parameter("out", [PER, H], F32, isOutput=True)

    with tile.TileContext(nc) as tc:
        with (
            tc.tile_pool(name="const", bufs=1) as cpool,
            tc.tile_pool(name="w", bufs=1) as wpool,
            tc.tile_pool(name="stage", bufs=1) as spool,
            tc.tile_pool(name="act", bufs=2) as apool,
            tc.tile_pool(name="pr", bufs=2, space="PSUM") as pr_pool,
            tc.tile_pool(name="pz", bufs=2, space="PSUM") as pz_pool,
            tc.tile_pool(name="pc", bufs=2, space="PSUM") as pc_pool,
            tc.tile_pool(name="pt", bufs=2, space="PSUM") as pt_pool,
        ):
            ident = cpool.tile([128, 128], F32)
            make_identity(nc, ident[:])

            starts = np.concatenate([[0], np.cumsum(CHUNKS)]).astype(int)
            st = {}

            # single line-rate aux transfer, issued before everything
            aux = spool.tile([128, 4 * PER], F32, tag="aux")
            nc.gpsimd.dma_start(
                out=aux[:],
                in_=aux_d.rearrange("(p g) -> p g", p=128),
            )

            def ablk(k, i):
                off = 4 * int(starts[k]) + i * CHUNKS[k]
                return aux[:, off:off + CHUNKS[k]]

            def issue_dma(k):
                G = CHUNKS[k]
                G2 = G // 2
                wq = nc.sync if k % 2 == 0 else nc.scalar
                w8_off = int(starts[k]) * 2 * C * 64
                w8 = wpool.tile([128, G2, 128], FP8, tag=f"w8{k}")
                (nc.sync if k == LAST else wq).dma_start(
                    out=w8[:],
                    in_=w8_d[w8_off:w8_off + 128 * G2 * 128].rearrange(
                        "(c t j) -> c t j", c=128, t=G2),
                )
                w16_off = int(starts[k]) * 2 * C * 128
                if k == LAST:
                    half = G2 // 2
                    wa = wpool.tile([128, half, 256], BF16, tag=f"w16{k}a")
                    wb = wpool.tile([128, G2 - half, 256], BF16,
                                    tag=f"w16{k}b")
                    wsrc = w16_d[w16_off:w16_off + 128 * G2 * 256].rearrange(
                        "(c t j) -> c t j", c=128, t=G2)
                    nc.sync.dma_start(out=wa[:], in_=wsrc[:, 0:half, :])
                    nc.scalar.dma_start(out=wb[:], in_=wsrc[:, half:G2, :])
                    st[k] = {"w16": (wa, wb, half), "w8": w8}
                else:
                    w16 = wpool.tile([128, G2, 256], BF16, tag=f"w16{k}")
                    wq.dma_start(
                        out=w16[:],
                        in_=w16_d[w16_off:w16_off + 128 * G2 * 256].rearrange(
                            "(c t j) -> c t j", c=128, t=G2),
                    )
                    st[k] = {"w16": (w16, None, G2), "w8": w8}

            def w16slice(k, t):
                wa, wb, half = st[k]["w16"]
                if wb is None or t < half:
                    return wa[:, t]
                return wb[:, t - half]

            def issue_cast(k):
                G = CHUNKS[k]
                xh32 = ablk(k, 0)
                xh_m = apool.tile([128, GMAX], BF16, tag="xh_m")
                nc.vector.tensor_copy(xh_m[:, 0:G], xh32)
                xh8 = apool.tile([128, GMAX], FP8, tag="xh8")
                nc.vector.tensor_copy(xh8[:, 0:G], xh32)
                st[k].update(xh_m=xh_m, xh8=xh8)

            def issue_rz(k):
                G = CHUNKS[k]
                G2 = G // 2
                w8 = st[k]["w8"]
                xh_m, xh8 = st[k]["xh_m"], st[k]["xh8"]
                psum_r = pr_pool.tile([128, GMAX], F32, tag="r")
                for t in range(G2):
                    nc.tensor.matmul(
                        psum_r[:, 2 * t:2 * t + 2],
                        w8[:, t, 0:128],
                        xh8[:, 2 * t:2 * t + 2],
                        start=True, stop=True,
                    )
                psum_z = pz_pool.tile([128, GMAX], F32, tag="z")
                for t in range(G2):
                    nc.tensor.matmul(
                        psum_z[:, 2 * t:2 * t + 2],
                        w16slice(k, t)[:, 0:128],
                        xh_m[:, 2 * t:2 * t + 2],
                        start=True, stop=True,
                    )
                st[k]["psum_r"] = psum_r
                st[k]["psum_z"] = psum_z

            def issue_epi1(k):
                G = CHUNKS[k]
                xh_m = st[k]["xh_m"]
                psum_r, psum_z = st[k]["psum_r"], st[k]["psum_z"]
                t_r = apool.tile([128, GMAX], F32, tag="t_r")
                nc.vector.tensor_add(t_r[:, 0:G], psum_r[:, 0:G], ablk(k, 1))
                t_z = apool.tile([128, GMAX], F32, tag="t_z")
                nc.vector.tensor_add(t_z[:, 0:G], psum_z[:, 0:G], ablk(k, 2))
                rs = apool.tile([128, GMAX], BF16, tag="rs")
                nc.scalar.activation(rs[64:128, 0:G:2], t_r[0:64, 0:G:2],
                                     AF.Sigmoid)
                nc.scalar.activation(rs[64:128, 1:G:2], t_r[64:128, 1:G:2],
                                     AF.Sigmoid)
                zs = apool.tile([128, GMAX], F32, tag="zs")
                nc.scalar.activation(zs[64:128, 0:G:2], t_z[0:64, 0:G:2],
                                     AF.Sigmoid)
                nc.scalar.activation(zs[64:128, 1:G:2], t_z[64:128, 1:G:2],
                                     AF.Sigmoid)
                rhs2 = apool.tile([128, GMAX], BF16, tag="rhs2")
                nc.vector.tensor_copy(rhs2[0:64, 0:G], xh_m[0:64, 0:G])
                nc.vector.tensor_mul(rhs2[64:128, 0:G], rs[64:128, 0:G],
                                     xh_m[64:128, 0:G])
                st[k].update(zs=zs, rhs2=rhs2)

            def issue_c(k):
                G = CHUNKS[k]
                G2 = G // 2
                rhs2 = st[k]["rhs2"]
                psum_c = pc_pool.tile([128, GMAX], F32, tag="c")
                for t in range(G2):
                    nc.tensor.matmul(
                        psum_c[:, 2 * t:2 * t + 2],
                        w16slice(k, t)[:, 128:256],
                        rhs2[:, 2 * t:2 * t + 2],
                        start=True, stop=True,
                    )
                st[k]["psum_c"] = psum_c

            def issue_epi2(k):
                G = CHUNKS[k]
                zs, psum_c = st[k]["zs"], st[k]["psum_c"]
                h32 = ablk(k, 0)  # rows 64:128 hold h in f32
                bct = ablk(k, 3)
                t_c = apool.tile([128, GMAX], F32, tag="t_c")
                nc.vector.tensor_add(t_c[0:64, 0:G:2], psum_c[0:64, 0:G:2],
                                     bct[0:64, 0:G:2])
                nc.vector.tensor_add(t_c[64:128, 1:G:2], psum_c[64:128, 1:G:2],
                                     bct[64:128, 1:G:2])
                hc = apool.tile([128, GMAX], F32, tag="hc")
                nc.scalar.activation(hc[64:128, 0:G:2], t_c[0:64, 0:G:2],
                                     AF.Tanh)
                nc.scalar.activation(hc[64:128, 1:G:2], t_c[64:128, 1:G:2],
                                     AF.Tanh)
                # h_new = h + z*(hc - h)
                e = apool.tile([128, GMAX], F32, tag="e")
                nc.vector.tensor_sub(e[64:128, 0:G], hc[64:128, 0:G],
                                     h32[64:128, :])
                ze = apool.tile([128, GMAX], F32, tag="ze")
                nc.vector.tensor_mul(ze[64:128, 0:G], zs[64:128, 0:G],
                                     e[64:128, 0:G])
                hn = apool.tile([128, GMAX], F32, tag="hn")
                nc.vector.tensor_add(hn[64:128, 0:G], h32[64:128, :],
                                     ze[64:128, 0:G])
                st[k]["hn"] = hn

            def issue_out_T(k):
                G = CHUNKS[k]
                hn = st[k]["hn"]
                p_t = pt_pool.tile([128, 64], F32, tag="t")
                nc.tensor.transpose(p_t[0:G, :], hn[64:128, 0:G],
                                    ident[64:128, 64:128])
                ot = spool.tile([G, 64], F32, tag=f"ot{k}")
                nc.scalar.activation(ot[:], p_t[0:G, :], AF.Copy)
                oq = nc.scalar if k == LAST else nc.gpsimd
                oq.dma_start(out=out_d[starts[k]:starts[k] + G], in_=ot[:])

            for k in range(NCHUNK):
                issue_dma(k)
            for k in range(NCHUNK):
                issue_cast(k)
                issue_rz(k)
                issue_epi1(k)
                if k - 2 >= 0:
                    issue_out_T(k - 2)
                if k - 1 >= 0:
                    issue_c(k - 1)
                    issue_epi2(k - 1)
            issue_c(LAST)
            issue_epi2(LAST)
            issue_out_T(LAST - 1)
            issue_out_T(LAST)

    nc.compile()
    return nc


_CACHE = {}


def _get_nc():
    if "nc" not in _CACHE:
        _CACHE["nc"] = build_nc()
    return _CACHE["nc"]


def _shards(x, state, Wx, Wh, b):
    import ml_dtypes
    F8NP = ml_dtypes.float8_e4m3
    x2 = np.asarray(x, np.float32).reshape(ITEMS, C)
    h2 = np.asarray(state, np.float32).reshape(ITEMS, H)
    xhf = np.concatenate([x2, h2], axis=1).reshape(NCORES, PER, 2 * C)
    b2 = np.asarray(b, np.float32).reshape(NCORES, PER, J)
    wx2 = np.asarray(Wx).reshape(ITEMS, C, J)
    wh2 = np.asarray(Wh).reshape(ITEMS, H, J)
    wf = np.concatenate([wx2, wh2], axis=1)          # [items, 128, 192]
    wr = wf[:, :, 0:64].astype(F8NP).reshape(NCORES, PER, 2 * C, 64)
    wzc = np.concatenate([wf[:, :, 64:128], wf[:, :, 128:192]],
                         axis=2).astype(ml_dtypes.bfloat16)
    wzc = wzc.reshape(NCORES, PER, 2 * C, 128)       # z cols 0:64, c 64:128
    maps = []
    for i in range(NCORES):
        w8b, w16b, ab = [], [], []
        s = 0
        for G in CHUNKS:
            r = wr[i, s:s + G]                       # [G, 128, 64]
            blk8 = np.concatenate([r[0::2], r[1::2]], axis=2)  # [G2,128,128]
            w8b.append(np.ascontiguousarray(
                blk8.transpose(1, 0, 2)).ravel())
            zc = wzc[i, s:s + G]                     # [G, 128, 128]
            a, o = zc[0::2], zc[1::2]
            blk16 = np.concatenate(
                [a[:, :, 0:64], o[:, :, 0:64],
                 a[:, :, 64:128], o[:, :, 64:128]], axis=2)    # [G2,128,256]
            w16b.append(np.ascontiguousarray(
                blk16.transpose(1, 0, 2)).ravel())
            bg = b2[i, s:s + G]                      # [G, 192]
            br = np.zeros((128, G), np.float32)
            br[0:64, 0::2] = bg[0::2, 0:64].T
            br[64:128, 1::2] = bg[1::2, 0:64].T
            bz = np.zeros((128, G), np.float32)
            bz[0:64, 0::2] = bg[0::2, 64:128].T
            bz[64:128, 1::2] = bg[1::2, 64:128].T
            bct = np.concatenate([bg[:, 128:192].T, bg[:, 128:192].T], axis=0)
            ab.append(np.concatenate(
                [xhf[i, s:s + G].T, br, bz, bct], axis=1))     # [128, 4G]
            s += G
        maps.append({
            "aux": np.ascontiguousarray(
                np.concatenate(ab, axis=1)).ravel(),
            "w8": np.concatenate(w8b),
            "w16": np.concatenate(w16b),
        })
    return maps


def kernel(x, state, Wx, Wh, b, _trace=False):
    nc = _get_nc()
    in_maps = _shards(x, state, Wx, Wh, b)
    res = run_bass_kernel_spmd(nc, in_maps, list(range(NCORES)), trace=_trace)
    out = np.concatenate([res.results[i]["out"] for i in range(NCORES)], axis=0)
    ret = out.reshape(B, N, 1, H).astype(np.float32)
    if _trace:
        return ret, res
    return ret


if __name__ == "__main__":
    build_nc()
    print("build OK")


# revision 3
# speedup vs baseline: 1.0750x; 1.0750x over previous
"""Trainium2 Bass kernel for batched per-item GRU cell (final).

v7 + instruction/semaphore-count and tail reductions:

  * ONE weight DMA per chunk: the fp8 r-pair block is bitcast into the
    bf16 weight param (per pair 320 bf16 cols = [r_pair fp8 x128 (64) |
    z_pair (128) | c_pair (128)]).  9 weight DMAs total (last chunk
    split across both rings).
  * r and z matmuls share ONE PSUM tile [128, 2G] (r diag at cols 0:G,
    z diag at G:2G): one DVE bias add and TWO parity sigmoids replace
    v7's two adds + four sigmoids; rs/zs are slices of one tile.
  * aux ([xh32 | brz_diag(2G) | bct] per chunk, f32) rides the scalar
    HWDGE ring FIRST — lands ~10us, PE starts ~12us.
  * Tail-light chunk schedule [16,64,96,90,80,36,24,8]; the last three
    output stores go on the (idle) HWDGE rings instead of SWDGE.
"""

import numpy as np

import concourse.bass as bass
import concourse.mybir as mybir
import concourse.tile as tile
from concourse import bacc
from concourse.bass_utils import run_bass_kernel_spmd

F32 = mybir.dt.float32
BF16 = mybir.dt.bfloat16
FP8 = mybir.dt.float8e4

B, N, C, H = 16, 207, 64, 64
J = 3 * H                  # 192
ITEMS = B * N              # 3312
NCORES = 8
PER = ITEMS // NCORES      # 414
CHUNKS = [16, 64, 96, 90, 80, 36, 24, 8]   # sum=414, all even
NCHUNK = len(CHUNKS)
GMAX = max(CHUNKS)
LAST = NCHUNK - 1
WPP = 320                  # bf16 cols per pair: 64 (fp8 r x2) +128 z +128 c

AF = mybir.ActivationFunctionType


def build_nc():
    nc = bacc.Bacc(None)
    # per chunk [128, 4G] f32: [xh32(G) | brz_diag(2G) | bct(G)]
    aux_d = nc.declare_dram_parameter("aux", [128 * 4 * PER], F32,
                                      isOutput=False)
    w_d = nc.declare_dram_parameter("w", [PER // 2 * 128 * WPP], BF16,
                                    isOutput=False)
    # transposed output blocks per chunk: [64, G] (host un-transposes)
    out_d = nc.declare_dram_parameter("out", [H * PER], F32, isOutput=True)

    with tile.TileContext(nc) as tc:
        with (
            tc.tile_pool(name="w", bufs=1) as wpool,
            tc.tile_pool(name="stage", bufs=1) as spool,
            tc.tile_pool(name="act", bufs=2) as apool,
            tc.tile_pool(name="prz", bufs=2, space="PSUM") as prz_pool,
            tc.tile_pool(name="pc", bufs=2, space="PSUM") as pc_pool,
        ):
            # aux first on the scalar ring: lands before the first chunks
            aux = spool.tile([128, 4 * PER], F32, tag="aux")
            nc.scalar.dma_start(
                out=aux[:],
                in_=aux_d.rearrange("(p g) -> p g", p=128),
            )
            starts = np.concatenate([[0], np.cumsum(CHUNKS)]).astype(int)
            st = {}

            def ablk(k, i, n=1):
                off = 4 * int(starts[k]) + i * CHUNKS[k]
                return aux[:, off:off + n * CHUNKS[k]]

            def issue_dma(k):
                G = CHUNKS[k]
                G2 = G // 2
                w_off = int(starts[k]) // 2 * 128 * WPP
                if k == LAST:
                    half = G2 // 2
                    wa = wpool.tile([128, half, WPP], BF16, tag=f"w{k}a")
                    wb = wpool.tile([128, G2 - half, WPP], BF16, tag=f"w{k}b")
                    wsrc = w_d[w_off:w_off + 128 * G2 * WPP].rearrange(
                        "(c t j) -> c t j", c=128, t=G2)
                    nc.sync.dma_start(out=wa[:], in_=wsrc[:, 0:half, :])
                    nc.scalar.dma_start(out=wb[:], in_=wsrc[:, half:G2, :])
                    st[k] = {"w": (wa, wb, half)}
                else:
                    wq = nc.sync if k % 2 == 0 else nc.scalar
                    w = wpool.tile([128, G2, WPP], BF16, tag=f"w{k}")
                    wq.dma_start(
                        out=w[:],
                        in_=w_d[w_off:w_off + 128 * G2 * WPP].rearrange(
                            "(c t j) -> c t j", c=128, t=G2),
                    )
                    st[k] = {"w": (w, None, G2)}

            def wslice(k, t):
                wa, wb, half = st[k]["w"]
                if wb is None or t < half:
                    return wa[:, t]
                return wb[:, t - half]

            def issue_cast(k):
                G = CHUNKS[k]
                xh32 = ablk(k, 0)
                xh_m = apool.tile([128, GMAX], BF16, tag="xh_m")
                nc.vector.tensor_copy(xh_m[:, 0:G], xh32)
                xh8 = apool.tile([128, GMAX], FP8, tag="xh8")
                nc.vector.tensor_copy(xh8[:, 0:G], xh32)
                rhs2 = apool.tile([128, GMAX], BF16, tag="rhs2")
                nc.vector.tensor_copy(rhs2[0:64, 0:G], xh32[0:64, :])
                st[k].update(xh_m=xh_m, xh8=xh8, rhs2=rhs2)

            def issue_rz(k):
                G = CHUNKS[k]
                G2 = G // 2
                xh_m, xh8 = st[k]["xh_m"], st[k]["xh8"]
                psum_rz = prz_pool.tile([128, 2 * GMAX], F32, tag="rz")
                for t in range(G2):
                    nc.tensor.matmul(
                        psum_rz[:, 2 * t:2 * t + 2],
                        wslice(k, t)[:, 0:64].bitcast(FP8),
                        xh8[:, 2 * t:2 * t + 2],
                        start=True, stop=True,
                    )
                for t in range(G2):
                    nc.tensor.matmul(
                        psum_rz[:, G + 2 * t:G + 2 * t + 2],
                        wslice(k, t)[:, 64:192],
                        xh_m[:, 2 * t:2 * t + 2],
                        start=True, stop=True,
                    )
                st[k]["psum_rz"] = psum_rz

            def issue_epi1(k):
                G = CHUNKS[k]
                xh32 = ablk(k, 0)
                psum_rz = st[k]["psum_rz"]
                t_rz = apool.tile([128, 2 * GMAX], F32, tag="t_rz")
                nc.vector.tensor_add(t_rz[:, 0:2 * G], psum_rz[:, 0:2 * G],
                                     ablk(k, 1, 2))
                # r/z sigmoid by parity: evens on rows 0:64, odds 64:128
                sig = apool.tile([128, 2 * GMAX], F32, tag="sig")
                nc.scalar.activation(sig[64:128, 0:2 * G:2],
                                     t_rz[0:64, 0:2 * G:2], AF.Sigmoid)
                nc.scalar.activation(sig[64:128, 1:2 * G:2],
                                     t_rz[64:128, 1:2 * G:2], AF.Sigmoid)
                rhs2 = st[k]["rhs2"]
                nc.vector.tensor_mul(rhs2[64:128, 0:G], sig[64:128, 0:G],
                                     xh32[64:128, :])
                # u = (1-z)*h, ready before the c matmuls finish
                zs = sig[:, G:2 * G]
                zh = apool.tile([128, GMAX], F32, tag="zh")
                nc.vector.tensor_mul(zh[64:128, 0:G], zs[64:128, :],
                                     xh32[64:128, :])
                u = apool.tile([128, GMAX], F32, tag="u")
                nc.vector.tensor_sub(u[64:128, 0:G], xh32[64:128, :],
                                     zh[64:128, 0:G])
                st[k].update(sig=sig, u=u)

            def issue_c(k):
                G = CHUNKS[k]
                G2 = G // 2
                rhs2 = st[k]["rhs2"]
                psum_c = pc_pool.tile([128, GMAX], F32, tag="c")
                for t in range(G2):
                    nc.tensor.matmul(
                        psum_c[:, 2 * t:2 * t + 2],
                        wslice(k, t)[:, 192:320],
                        rhs2[:, 2 * t:2 * t + 2],
                        start=True, stop=True,
                    )
                st[k]["psum_c"] = psum_c

            def issue_epi2(k):
                G = CHUNKS[k]
                sig, psum_c = st[k]["sig"], st[k]["psum_c"]
                u = st[k]["u"]
                bct = ablk(k, 3)
                zs = sig[:, G:2 * G]
                t_c = apool.tile([128, GMAX], F32, tag="t_c")
                nc.vector.tensor_add(t_c[0:64, 0:G:2], psum_c[0:64, 0:G:2],
                                     bct[0:64, 0:G:2])
                nc.vector.tensor_add(t_c[64:128, 1:G:2], psum_c[64:128, 1:G:2],
                                     bct[64:128, 1:G:2])
                hc = apool.tile([128, GMAX], F32, tag="hc")
                nc.scalar.activation(hc[64:128, 0:G:2], t_c[0:64, 0:G:2],
                                     AF.Tanh)
                nc.scalar.activation(hc[64:128, 1:G:2], t_c[64:128, 1:G:2],
                                     AF.Tanh)
                # h_new = (1-z)*h + z*hc, with u=(1-z)*h precomputed
                zhc = apool.tile([128, GMAX], F32, tag="zhc")
                nc.vector.tensor_mul(zhc[64:128, 0:G], zs[64:128, :],
                                     hc[64:128, 0:G])
                hn = apool.tile([128, GMAX], F32, tag="hn")
                nc.vector.tensor_add(hn[64:128, 0:G], u[64:128, 0:G],
                                     zhc[64:128, 0:G])
                st[k]["hn"] = hn

            def issue_out_T(k):
                G = CHUNKS[k]
                hn = st[k]["hn"]
                # store the [64, G] column block directly; host transposes
                if k >= NCHUNK - 3:
                    oq = nc.scalar if k % 2 else nc.sync
                else:
                    oq = nc.gpsimd
                o_off = int(starts[k]) * H
                oq.dma_start(
                    out=out_d[o_off:o_off + H * G].rearrange(
                        "(p g) -> p g", p=H),
                    in_=hn[64:128, 0:G],
                )

            for k in range(NCHUNK):
                issue_dma(k)
            for k in range(NCHUNK):
                issue_cast(k)
                issue_rz(k)
                issue_epi1(k)
                if k - 2 >= 0:
                    issue_out_T(k - 2)
                if k - 1 >= 0:
                    issue_c(k - 1)
                    issue_epi2(k - 1)
            issue_c(LAST)
            issue_epi2(LAST)
            issue_out_T(LAST - 1)
            issue_out_T(LAST)

    nc.compile()
    return nc


_CACHE = {}


def _get_nc():
    if "nc" not in _CACHE:
        _CACHE["nc"] = build_nc()
    return _CACHE["nc"]


def _shards(x, state, Wx, Wh, b):
    import ml_dtypes
    F8NP = ml_dtypes.float8_e4m3
    BFNP = ml_dtypes.bfloat16
    x2 = np.asarray(x, np.float32).reshape(ITEMS, C)
    h2 = np.asarray(state, np.float32).reshape(ITEMS, H)
    xhf = np.concatenate([x2, h2], axis=1).reshape(NCORES, PER, 2 * C)
    b2 = np.asarray(b, np.float32).reshape(NCORES, PER, J)
    wx2 = np.asarray(Wx).reshape(ITEMS, C, J)
    wh2 = np.asarray(Wh).reshape(ITEMS, H, J)
    wf = np.concatenate([wx2, wh2], axis=1)          # [items, 128, 192]
    wr = wf[:, :, 0:64].astype(F8NP).reshape(NCORES, PER, 2 * C, 64)
    wz = wf[:, :, 64:128].astype(BFNP).reshape(NCORES, PER, 2 * C, 64)
    wc = wf[:, :, 128:192].astype(BFNP).reshape(NCORES, PER, 2 * C, 64)
    maps = []
    for i in range(NCORES):
        wb, ab = [], []
        s = 0
        for G in CHUNKS:
            G2 = G // 2
            r = wr[i, s:s + G]                       # [G, 128, 64] fp8
            rblk = np.concatenate([r[0::2], r[1::2]], axis=2)  # [G2,128,128]
            z = wz[i, s:s + G]
            zblk = np.concatenate([z[0::2], z[1::2]], axis=2)  # [G2,128,128]
            c = wc[i, s:s + G]
            cblk = np.concatenate([c[0::2], c[1::2]], axis=2)  # [G2,128,128]
            # bytes per pair row: 128 (fp8) + 256 (z bf16) + 256 (c bf16)
            byts = np.concatenate(
                [rblk.view(np.uint8),
                 zblk.view(np.uint8).reshape(G2, 128, 256),
                 cblk.view(np.uint8).reshape(G2, 128, 256)], axis=2)
            wb.append(np.ascontiguousarray(
                byts.transpose(1, 0, 2)).ravel().view(BFNP))   # [128,G2,640B]
            bg = b2[i, s:s + G]                      # [G, 192]
            brz = np.zeros((128, 2 * G), np.float32)
            brz[0:64, 0:G:2] = bg[0::2, 0:64].T      # r evens
            brz[64:128, 1:G:2] = bg[1::2, 0:64].T    # r odds
            brz[0:64, G::2] = bg[0::2, 64:128].T     # z evens
            brz[64:128, G + 1::2] = bg[1::2, 64:128].T
            bct = np.concatenate([bg[:, 128:192].T, bg[:, 128:192].T], axis=0)
            ab.append(np.concatenate(
                [xhf[i, s:s + G].T, brz, bct], axis=1))        # [128, 4G]
            s += G
        maps.append({
            "aux": np.ascontiguousarray(np.concatenate(ab, axis=1)).ravel(),
            "w": np.concatenate(wb),
        })
    return maps


def kernel(x, state, Wx, Wh, b, _trace=False):
    nc = _get_nc()
    in_maps = _shards(x, state, Wx, Wh, b)
    res = run_bass_kernel_spmd(nc, in_maps, list(range(NCORES)), trace=_trace)
    blocks = []
    for i in range(NCORES):
        flat = res.results[i]["out"]
        s = 0
        for G in CHUNKS:
            blocks.append(flat[H * s:H * (s + G)].reshape(H, G).T)
            s += G
    out = np.concatenate(blocks, axis=0)
    ret = out.reshape(B, N, 1, H).astype(np.float32)
    if _trace:
        return ret, res
    return ret


if __name__ == "__main__":
    build_nc()
    print("build OK")
